# revision 6
# baseline (speedup 1.0000x reference)
"""Causal multi-head attention (B=2, H=16, S=2048, F=128) on 8 TRN2 NeuronCores.

Sharding: tensor-parallel over the (batch, head) axis — 32 independent
(b, h) attention problems, 4 per core. No collectives needed.

Per-head on-chip algorithm (all layouts chosen so no on-chip transposes
are ever required):
  - host pre-transposes x to xT [F, S] per head, and W to WT [f, e].
  - QT = WqT.T @ xT   (PSUM) + bias -> SBUF   [e=128, s=2048]
  - KT likewise.
  - V' = xT_tile.T @ [WvT | 0] + [bv | 1]     [s=128, e'=129] per s-tile
    (extra ones column makes the AV matmul also produce softmax denominators)
  - For each k-tile (128 keys), compute ST = K_tile . Q  ->  [k=128, q<=1024]
    strips in PSUM, exp on ACT -> PT (bf16) in SBUF, lower-triangle zero-mask
    on the diagonal block, then AV: out_acc[q,129] += PT_tile.T @ V'_tile,
    accumulated in PSUM over k-tiles. Column 128 of the accumulator is
    sum(exp) — normalize with DVE reciprocal + tensor_scalar multiply.
  - Causality: only k-tiles with k <= q are ever computed.
"""

import math

import numpy as np
import ml_dtypes

import concourse.tile as tile
import concourse.mybir as mybir
from concourse import bacc, bass_utils

B, H, S, F = 2, 16, 2048, 128
NCORES = 8
HPC = (B * H) // NCORES  # (b,h) pairs per core
SCALE = 1.0 / math.sqrt(F)
HALF = S // 2  # q processed in two 1024-wide halves (PSUM budget)
GSTRIDE = 136  # col stride of packed AV accumulator groups (32B aligned)

QK_FP32R = True  # scores/projection matmuls in fp32r (else bf16)

_cache = {}


def _build():
    f32 = mybir.dt.float32
    bf16 = mybir.dt.bfloat16
    f32r = mybir.dt.float32r
    qk_dt = f32r if QK_FP32R else bf16
    Exp = mybir.ActivationFunctionType.Exp

    nc = bacc.Bacc("TRN2")

    if QK_FP32R:
        xtq = nc.dram_tensor("xt32", [HPC, F, S], f32r, kind="ExternalInput")
    xtb = nc.dram_tensor("xtbh", [HPC, F, S], bf16, kind="ExternalInput")
    wqt = nc.dram_tensor("wqt", [HPC, F, F], qk_dt, kind="ExternalInput")
    wkt = nc.dram_tensor("wkt", [HPC, F, F], qk_dt, kind="ExternalInput")
    wvt = nc.dram_tensor("wvt", [HPC, F, F + 1], bf16, kind="ExternalInput")
    bqt = nc.dram_tensor("bqt", [F, HPC], f32, kind="ExternalInput")
    bkt = nc.dram_tensor("bkt", [F, HPC], f32, kind="ExternalInput")
    bvp = nc.dram_tensor("bvp", [1, HPC * (F + 1)], bf16, kind="ExternalInput")
    msk = nc.dram_tensor("msk", [F, F], bf16, kind="ExternalInput")
    one = nc.dram_tensor("one", [1, F], bf16, kind="ExternalInput")
    out = nc.dram_tensor("out", [HPC, S, F], f32, kind="ExternalOutput")
    if not QK_FP32R:
        xtq = xtb

    with tile.TileContext(nc) as tc, \
            tc.tile_pool(name="consts", bufs=1) as consts, \
            tc.tile_pool(name="xin", bufs=2) as xin, \
            tc.tile_pool(name="qk", bufs=2) as qkp, \
            tc.tile_pool(name="vp", bufs=2 * (S // F)) as vpp, \
            tc.tile_pool(name="pt", bufs=3) as ptp, \
            tc.tile_pool(name="outs", bufs=6) as outp, \
            tc.tile_pool(name="st", bufs=2, space="PSUM") as stp, \
            tc.tile_pool(name="av", bufs=4, space="PSUM") as avp:

        c_bq = consts.tile([F, HPC], f32, tag="bq")
        nc.sync.dma_start(out=c_bq, in_=bqt[:, :])
        c_bk = consts.tile([F, HPC], f32, tag="bk")
        nc.sync.dma_start(out=c_bk, in_=bkt[:, :])
        c_bv = consts.tile([1, HPC * (F + 1)], bf16, tag="bv")
        nc.sync.dma_start(out=c_bv, in_=bvp[:, :])
        c_mask = consts.tile([F, F], bf16, tag="msk")
        nc.sync.dma_start(out=c_mask, in_=msk[:, :])
        c_one = consts.tile([1, F], bf16, tag="one")
        nc.sync.dma_start(out=c_one, in_=one[:, :])

        # deferred AV-batch emission: keeps PE busy with the next ST strip
        # while ACT runs exp on the current one
        pending = []

        def flush_pending():
            while pending:
                pending.pop(0)()

        for hd in range(HPC):
            if QK_FP32R:
                x32 = xin.tile([F, S], f32r, tag="x32")
                nc.sync.dma_start(out=x32, in_=xtq[hd])
            xbh = xin.tile([F, S], bf16, tag="xbh")
            nc.sync.dma_start(out=xbh, in_=xtb[hd])
            xqk = x32 if QK_FP32R else xbh
            wq = xin.tile([F, F], qk_dt, tag="wq")
            nc.sync.dma_start(out=wq, in_=wqt[hd])
            wk = xin.tile([F, F], qk_dt, tag="wk")
            nc.sync.dma_start(out=wk, in_=wkt[hd])
            wv = xin.tile([F, F + 1], bf16, tag="wv")
            nc.sync.dma_start(out=wv, in_=wvt[hd])

            # --- QT / KT projections (e on partitions) ---
            qt_t = qkp.tile([F, S], qk_dt, tag="qt")
            kt_t = qkp.tile([F, S], qk_dt, tag="kt")
            for wt, bt, dst in ((wq, c_bq, qt_t), (wk, c_bk, kt_t)):
                for c in range(S // 512):
                    ps = stp.tile([128, 1024], f32, tag="st")
                    nc.tensor.matmul(
                        ps[:, 0:512], wt[:, :],
                        xqk[:, 512 * c:512 * (c + 1)],
                        start=True, stop=True)
                    nc.vector.tensor_scalar_add(
                        dst[:, 512 * c:512 * (c + 1)], ps[:, 0:512],
                        bt[:, hd:hd + 1])

            flush_pending()  # prev head's AV tail runs under this head's QKV

            # --- V' projection (s on partitions, ones column fused) ---
            vtiles = []
            for si in range(S // 128):
                ps = avp.tile([128, 512], f32, tag="av")
                nc.tensor.matmul(
                    ps[:, 0:F + 1], c_one[:, :],
                    c_bv[:, (F + 1) * hd:(F + 1) * (hd + 1)],
                    start=True, stop=False)
                nc.tensor.matmul(
                    ps[:, 0:F + 1], xbh[:, 128 * si:128 * (si + 1)], wv[:, :],
                    start=False, stop=True)
                vt = vpp.tile([128, F + 1], bf16, tag="vp")
                nc.vector.tensor_copy(out=vt[:, :], in_=ps[:, 0:F + 1])
                vtiles.append(vt)

            # --- attention, q in two 1024-wide halves ---
            for half in range(2):
                q0 = half * HALF
                nk = (half + 1) * (HALF // 128)  # k-tiles touching this half
                avts = [avp.tile([128, 512], f32, tag="av",
                                 name=f"avacc_{hd}_{half}_{i}")
                        for i in range(3)]
                # start=True clears has_written for the WHOLE bank, so a
                # per-group start would clobber the other groups packed in
                # the same bank. Clear each bank once with a dummy matmul
                # into a spare column; all real AV matmuls use start=False
                # (first write per element overwrites since its bit is clear).
                for b in range(3):
                    nc.tensor.matmul(
                        avts[b][:, 508:509], c_one[:, :], c_one[:, 0:1],
                        start=True, stop=False, skip_group_check=True)

                for ki in range(nk):
                    ks = 128 * ki
                    ls = max(0, ks - q0)  # local start col within strip
                    strip = stp.tile([128, 1024], f32, tag="st")
                    bounds = [ls, 512, 1024] if ls < 512 else [ls, 1024]
                    for c0, c1 in zip(bounds[:-1], bounds[1:]):
                        nc.tensor.matmul(
                            strip[:, c0:c1], kt_t[:, ks:ks + 128],
                            qt_t[:, q0 + c0:q0 + c1],
                            start=True, stop=True)
                    ptile = ptp.tile([128, 1024], bf16, tag="pt")
                    nc.scalar.activation(
                        out=ptile[:, ls:1024], in_=strip[:, ls:1024],
                        func=Exp, scale=SCALE)
                    if ks >= q0:  # zero the below-diagonal of the diag block
                        nc.vector.tensor_mul(
                            ptile[:, ls:ls + 128], ptile[:, ls:ls + 128],
                            c_mask[:, :])

                    def av_batch(hd=hd, half=half, ki=ki, ptile=ptile,
                                 avts=avts, vtiles=vtiles):
                        for qt in range(max(0, ki - 8 * half), 8):
                            qg = 8 * half + qt
                            g = GSTRIDE * (qt % 3)
                            acc = avts[qt // 3][:, g:g + F + 1]
                            nc.tensor.matmul(
                                acc, ptile[:, 128 * qt:128 * qt + 128],
                                vtiles[ki][:, :],
                                start=False, stop=(ki == qg),
                                skip_group_check=True)
                        # normalize + store once a whole accumulator bank
                        # is finished (avoids PE-write/DVE-read bank overlap)
                        for bank in range(3):
                            last_qt = min(3 * bank + 2, 7)
                            if ki != 8 * half + last_qt:
                                continue
                            for qt in range(3 * bank, last_qt + 1):
                                qg = 8 * half + qt
                                g = GSTRIDE * (qt % 3)
                                acc = avts[bank][:, g:g + F + 1]
                                rc = outp.tile([128, 1], f32, tag="rc")
                                nc.vector.reciprocal(rc[:, :], acc[:, F:F + 1])
                                ot = outp.tile([128, F], f32, tag="ot")
                                nc.vector.tensor_scalar_mul(
                                    ot[:, :], acc[:, 0:F], rc[:, :])
                                nc.sync.dma_start(
                                    out=out[hd, 128 * qg:128 * (qg + 1), :],
                                    in_=ot[:, :])

                    flush_pending()
                    pending.append(av_batch)
        flush_pending()

    nc.compile()
    return nc


def _prep_inputs(x, Wq, Wk, Wv, bq, bk, bv):
    """Shard + pre-transpose on host. Returns in_maps for 8 cores."""
    bf16 = ml_dtypes.bfloat16
    xf = np.ascontiguousarray(
        x.reshape(B * H, S, F).transpose(0, 2, 1)).astype(np.float32)  # [32,F,S]
    xfb = xf.astype(bf16)
    wqT = np.ascontiguousarray(Wq.transpose(0, 2, 1)).astype(np.float32)  # [H,f,e]
    wkT = np.ascontiguousarray(Wk.transpose(0, 2, 1)).astype(np.float32)
    wvT = np.ascontiguousarray(Wv.transpose(0, 2, 1)).astype(np.float32)
    wvTp = np.zeros((H, F, F + 1), np.float32)
    wvTp[:, :, :F] = wvT
    wvTp = wvTp.astype(bf16)
    bvp_h = np.concatenate(
        [bv.astype(np.float32), np.ones((H, 1), np.float32)], axis=1)  # [H,129]
    mask = np.triu(np.ones((F, F), np.float32)).astype(bf16)  # keep r <= c
    ones_row = np.ones((1, F), np.float32).astype(bf16)

    wq_dt = np.float32 if QK_FP32R else bf16
    in_maps = []
    for c in range(NCORES):
        pairs = list(range(HPC * c, HPC * (c + 1)))
        heads = [p % H for p in pairs]
        m = {
            "xtbh": np.ascontiguousarray(xfb[pairs]),
            "wqt": np.ascontiguousarray(wqT[heads]).astype(wq_dt),
            "wkt": np.ascontiguousarray(wkT[heads]).astype(wq_dt),
            "wvt": np.ascontiguousarray(wvTp[heads]),
            "bqt": np.ascontiguousarray(bq[heads].T).astype(np.float32),
            "bkt": np.ascontiguousarray(bk[heads].T).astype(np.float32),
            "bvp": np.ascontiguousarray(
                bvp_h[heads].reshape(1, HPC * (F + 1))).astype(bf16),
            "msk": mask,
            "one": ones_row,
        }
        if QK_FP32R:
            m["xt32"] = np.ascontiguousarray(xf[pairs])
        in_maps.append(m)
    return in_maps


def kernel(x, Wq, Wk, Wv, bq, bk, bv, trace=False):
    x, Wq, Wk, Wv = (np.asarray(a, np.float32) for a in (x, Wq, Wk, Wv))
    bq, bk, bv = (np.asarray(a, np.float32) for a in (bq, bk, bv))

    if "nc" not in _cache:
        _cache["nc"] = _build()
    nc = _cache["nc"]

    in_maps = _prep_inputs(x, Wq, Wk, Wv, bq, bk, bv)
    res = bass_utils.run_bass_kernel_spmd(
        nc, in_maps, core_ids=list(range(NCORES)), trace=trace)

    out = np.empty((B * H, S, F), np.float32)
    for c in range(NCORES):
        out[HPC * c:HPC * (c + 1)] = res.results[c]["out"]
    full = out.reshape(B, H, S, F)
    if trace:
        return full, res
    return full


# revision 11
# speedup vs baseline: 1.0817x; 1.0817x over previous
"""Causal multi-head attention (B=2, H=16, S=2048, F=128) on 8 TRN2 NeuronCores.

Sharding: tensor-parallel over the (batch, head) axis — 32 independent
(b, h) attention problems, 4 per core. No collectives needed.

Per-head on-chip algorithm (all layouts chosen so no on-chip transposes
are ever required):
  - host pre-transposes x to xT [F, S] per head, and W to WT [f, e].
  - QT = WqT.T @ xT   (PSUM) + bias -> SBUF   [e=128, s=2048]
  - KT likewise.
  - V' = xT_tile.T @ [WvT | 0] + [bv | 1]     [s=128, e'=129] per s-tile
    (extra ones column makes the AV matmul also produce softmax denominators)
  - For each k-tile (128 keys), compute ST = K_tile . Q  ->  [k=128, q<=1024]
    strips in PSUM, exp on ACT -> PT (bf16) in SBUF, lower-triangle zero-mask
    on the diagonal block, then AV: out_acc[q,129] += PT_tile.T @ V'_tile,
    accumulated in PSUM over k-tiles. Column 128 of the accumulator is
    sum(exp) — normalize with DVE reciprocal + tensor_scalar multiply.
  - Causality: only k-tiles with k <= q are ever computed.
"""

import math

import numpy as np
import ml_dtypes

import concourse.tile as tile
import concourse.mybir as mybir
from concourse import bacc, bass_utils

B, H, S, F = 2, 16, 2048, 128
NCORES = 8
HPC = (B * H) // NCORES  # (b,h) pairs per core
SCALE = 1.0 / math.sqrt(F)
HALF = S // 2  # q processed in two 1024-wide halves (PSUM budget)
GSTRIDE = 136  # col stride of packed AV accumulator groups (32B aligned)

QK_FP32R = False  # scores/projection matmuls in fp32r (else bf16)

_cache = {}


def _build():
    f32 = mybir.dt.float32
    bf16 = mybir.dt.bfloat16
    f32r = mybir.dt.float32r
    qk_dt = f32r if QK_FP32R else bf16
    Exp = mybir.ActivationFunctionType.Exp

    nc = bacc.Bacc("TRN2")

    if QK_FP32R:
        xtq = nc.dram_tensor("xt32", [HPC, F, S], f32r, kind="ExternalInput")
    xtb = nc.dram_tensor("xtbh", [HPC, F, S], bf16, kind="ExternalInput")
    wqt = nc.dram_tensor("wqt", [HPC, F, F], qk_dt, kind="ExternalInput")
    wkt = nc.dram_tensor("wkt", [HPC, F, F], qk_dt, kind="ExternalInput")
    wvt = nc.dram_tensor("wvt", [HPC, F, F + 1], bf16, kind="ExternalInput")
    bqt = nc.dram_tensor("bqt", [F, HPC], f32, kind="ExternalInput")
    bkt = nc.dram_tensor("bkt", [F, HPC], f32, kind="ExternalInput")
    bvp = nc.dram_tensor("bvp", [1, HPC * (F + 1)], bf16, kind="ExternalInput")
    msk = nc.dram_tensor("msk", [F, F], bf16, kind="ExternalInput")
    one = nc.dram_tensor("one", [1, F], bf16, kind="ExternalInput")
    out = nc.dram_tensor("out", [HPC, S, F], f32, kind="ExternalOutput")
    if not QK_FP32R:
        xtq = xtb

    with tile.TileContext(nc) as tc, \
            tc.tile_pool(name="consts", bufs=1) as consts, \
            tc.tile_pool(name="xin", bufs=2) as xin, \
            tc.tile_pool(name="qk", bufs=2) as qkp, \
            tc.tile_pool(name="vp", bufs=2 * (S // F)) as vpp, \
            tc.tile_pool(name="pt", bufs=3) as ptp, \
            tc.tile_pool(name="outs", bufs=6) as outp, \
            tc.tile_pool(name="st", bufs=2, space="PSUM") as stp, \
            tc.tile_pool(name="av", bufs=4, space="PSUM") as avp:

        c_bq = consts.tile([F, HPC], f32, tag="bq")
        nc.sync.dma_start(out=c_bq, in_=bqt[:, :])
        c_bk = consts.tile([F, HPC], f32, tag="bk")
        nc.sync.dma_start(out=c_bk, in_=bkt[:, :])
        c_bv = consts.tile([1, HPC * (F + 1)], bf16, tag="bv")
        nc.sync.dma_start(out=c_bv, in_=bvp[:, :])
        c_mask = consts.tile([F, F], bf16, tag="msk")
        nc.sync.dma_start(out=c_mask, in_=msk[:, :])
        c_one = consts.tile([1, F], bf16, tag="one")
        nc.sync.dma_start(out=c_one, in_=one[:, :])

        # deferred AV-batch emission: keeps PE busy with the next ST strip
        # while ACT runs exp on the current one
        pending = []

        def flush_pending():
            while pending:
                pending.pop(0)()

        for hd in range(HPC):
            if QK_FP32R:
                x32 = xin.tile([F, S], f32r, tag="x32")
                nc.sync.dma_start(out=x32, in_=xtq[hd])
            xbh = xin.tile([F, S], bf16, tag="xbh")
            nc.sync.dma_start(out=xbh, in_=xtb[hd])
            xqk = x32 if QK_FP32R else xbh
            wq = xin.tile([F, F], qk_dt, tag="wq")
            nc.sync.dma_start(out=wq, in_=wqt[hd])
            wk = xin.tile([F, F], qk_dt, tag="wk")
            nc.sync.dma_start(out=wk, in_=wkt[hd])
            wv = xin.tile([F, F + 1], bf16, tag="wv")
            nc.sync.dma_start(out=wv, in_=wvt[hd])

            # --- QT / KT projections (e on partitions) ---
            qt_t = qkp.tile([F, S], qk_dt, tag="qt")
            kt_t = qkp.tile([F, S], qk_dt, tag="kt")
            for wt, bt, dst in ((wq, c_bq, qt_t), (wk, c_bk, kt_t)):
                for c in range(S // 512):
                    ps = stp.tile([128, 1024], f32, tag="st")
                    nc.tensor.matmul(
                        ps[:, 0:512], wt[:, :],
                        xqk[:, 512 * c:512 * (c + 1)],
                        start=True, stop=True)
                    nc.vector.tensor_scalar_add(
                        dst[:, 512 * c:512 * (c + 1)], ps[:, 0:512],
                        bt[:, hd:hd + 1])

            flush_pending()  # prev head's AV tail runs under this head's QKV

            # --- V' projection (s on partitions, ones column fused) ---
            vtiles = []
            for si in range(S // 128):
                ps = avp.tile([128, 512], f32, tag="av")
                nc.tensor.matmul(
                    ps[:, 0:F + 1], c_one[:, :],
                    c_bv[:, (F + 1) * hd:(F + 1) * (hd + 1)],
                    start=True, stop=False)
                nc.tensor.matmul(
                    ps[:, 0:F + 1], xbh[:, 128 * si:128 * (si + 1)], wv[:, :],
                    start=False, stop=True)
                vt = vpp.tile([128, F + 1], bf16, tag="vp")
                nc.vector.tensor_copy(out=vt[:, :], in_=ps[:, 0:F + 1])
                vtiles.append(vt)

            # --- attention, q in two 1024-wide halves ---
            for half in range(2):
                q0 = half * HALF
                nk = (half + 1) * (HALF // 128)  # k-tiles touching this half
                avts = [avp.tile([128, 512], f32, tag="av",
                                 name=f"avacc_{hd}_{half}_{i}")
                        for i in range(3)]
                # start=True clears has_written for the WHOLE bank, so a
                # per-group start would clobber the other groups packed in
                # the same bank. Clear each bank once with a dummy matmul
                # into a spare column; all real AV matmuls use start=False
                # (first write per element overwrites since its bit is clear).
                for b in range(3):
                    nc.tensor.matmul(
                        avts[b][:, 508:509], c_one[:, :], c_one[:, 0:1],
                        start=True, stop=False, skip_group_check=True)

                for ki in range(nk):
                    ks = 128 * ki
                    ls = max(0, ks - q0)  # local start col within strip
                    strip = stp.tile([128, 1024], f32, tag="st")
                    bounds = [ls, 512, 1024] if ls < 512 else [ls, 1024]
                    for c0, c1 in zip(bounds[:-1], bounds[1:]):
                        nc.tensor.matmul(
                            strip[:, c0:c1], kt_t[:, ks:ks + 128],
                            qt_t[:, q0 + c0:q0 + c1],
                            start=True, stop=True)
                    ptile = ptp.tile([128, 1024], bf16, tag="pt")
                    nc.scalar.activation(
                        out=ptile[:, ls:1024], in_=strip[:, ls:1024],
                        func=Exp, scale=SCALE)
                    if ks >= q0:  # zero the below-diagonal of the diag block
                        nc.vector.tensor_mul(
                            ptile[:, ls:ls + 128], ptile[:, ls:ls + 128],
                            c_mask[:, :])

                    def av_batch(hd=hd, half=half, ki=ki, ptile=ptile,
                                 avts=avts, vtiles=vtiles):
                        for qt in range(max(0, ki - 8 * half), 8):
                            qg = 8 * half + qt
                            g = GSTRIDE * (qt % 3)
                            acc = avts[qt // 3][:, g:g + F + 1]
                            nc.tensor.matmul(
                                acc, ptile[:, 128 * qt:128 * qt + 128],
                                vtiles[ki][:, :],
                                start=False, stop=(ki == qg),
                                skip_group_check=True)
                        # normalize + store once a whole accumulator bank
                        # is finished (avoids PE-write/DVE-read bank overlap)
                        for bank in range(3):
                            last_qt = min(3 * bank + 2, 7)
                            if ki != 8 * half + last_qt:
                                continue
                            for qt in range(3 * bank, last_qt + 1):
                                qg = 8 * half + qt
                                g = GSTRIDE * (qt % 3)
                                acc = avts[bank][:, g:g + F + 1]
                                rc = outp.tile([128, 1], f32, tag="rc")
                                nc.vector.reciprocal(rc[:, :], acc[:, F:F + 1])
                                ot = outp.tile([128, F], f32, tag="ot")
                                nc.vector.tensor_scalar_mul(
                                    ot[:, :], acc[:, 0:F], rc[:, :])
                                nc.sync.dma_start(
                                    out=out[hd, 128 * qg:128 * (qg + 1), :],
                                    in_=ot[:, :])

                    flush_pending()
                    pending.append(av_batch)
        flush_pending()

    nc.compile()
    return nc


def _prep_inputs(x, Wq, Wk, Wv, bq, bk, bv):
    """Shard + pre-transpose on host. Returns in_maps for 8 cores."""
    bf16 = ml_dtypes.bfloat16
    xf = np.ascontiguousarray(
        x.reshape(B * H, S, F).transpose(0, 2, 1)).astype(np.float32)  # [32,F,S]
    xfb = xf.astype(bf16)
    wqT = np.ascontiguousarray(Wq.transpose(0, 2, 1)).astype(np.float32)  # [H,f,e]
    wkT = np.ascontiguousarray(Wk.transpose(0, 2, 1)).astype(np.float32)
    wvT = np.ascontiguousarray(Wv.transpose(0, 2, 1)).astype(np.float32)
    wvTp = np.zeros((H, F, F + 1), np.float32)
    wvTp[:, :, :F] = wvT
    wvTp = wvTp.astype(bf16)
    bvp_h = np.concatenate(
        [bv.astype(np.float32), np.ones((H, 1), np.float32)], axis=1)  # [H,129]
    mask = np.triu(np.ones((F, F), np.float32)).astype(bf16)  # keep r <= c
    ones_row = np.ones((1, F), np.float32).astype(bf16)

    wq_dt = np.float32 if QK_FP32R else bf16
    in_maps = []
    for c in range(NCORES):
        pairs = list(range(HPC * c, HPC * (c + 1)))
        heads = [p % H for p in pairs]
        m = {
            "xtbh": np.ascontiguousarray(xfb[pairs]),
            "wqt": np.ascontiguousarray(wqT[heads]).astype(wq_dt),
            "wkt": np.ascontiguousarray(wkT[heads]).astype(wq_dt),
            "wvt": np.ascontiguousarray(wvTp[heads]),
            "bqt": np.ascontiguousarray(bq[heads].T).astype(np.float32),
            "bkt": np.ascontiguousarray(bk[heads].T).astype(np.float32),
            "bvp": np.ascontiguousarray(
                bvp_h[heads].reshape(1, HPC * (F + 1))).astype(bf16),
            "msk": mask,
            "one": ones_row,
        }
        if QK_FP32R:
            m["xt32"] = np.ascontiguousarray(xf[pairs])
        in_maps.append(m)
    return in_maps


def kernel(x, Wq, Wk, Wv, bq, bk, bv, trace=False):
    x, Wq, Wk, Wv = (np.asarray(a, np.float32) for a in (x, Wq, Wk, Wv))
    bq, bk, bv = (np.asarray(a, np.float32) for a in (bq, bk, bv))

    if "nc" not in _cache:
        _cache["nc"] = _build()
    nc = _cache["nc"]

    in_maps = _prep_inputs(x, Wq, Wk, Wv, bq, bk, bv)
    res = bass_utils.run_bass_kernel_spmd(
        nc, in_maps, core_ids=list(range(NCORES)), trace=trace)

    out = np.empty((B * H, S, F), np.float32)
    for c in range(NCORES):
        out[HPC * c:HPC * (c + 1)] = res.results[c]["out"]
    full = out.reshape(B, H, S, F)
    if trace:
        return full, res
    return full


# revision 15
# speedup vs baseline: 1.1972x; 1.1067x over previous
"""Causal multi-head attention (B=2, H=16, S=2048, F=128) on 8 TRN2 NeuronCores.

Sharding: tensor-parallel over the (batch, head) axis — 32 independent
(b, h) attention problems, 4 per core. No collectives needed.

Per-head on-chip algorithm (all layouts chosen so no on-chip transposes
are ever required):
  - host pre-transposes x to xT [F, S] per head, and W to WT [f, e].
  - QT = WqT.T @ xT   (PSUM) + bias -> SBUF   [e=128, s=2048]
  - KT likewise.
  - V' = xT_tile.T @ [WvT | 0] + [bv | 1]     [s=128, e'=129] per s-tile
    (extra ones column makes the AV matmul also produce softmax denominators)
  - For each k-tile (128 keys), compute ST = K_tile . Q  ->  [k=128, q<=1024]
    strips in PSUM, exp on ACT -> PT (bf16) in SBUF, lower-triangle zero-mask
    on the diagonal block, then AV: out_acc[q,129] += PT_tile.T @ V'_tile,
    accumulated in PSUM over k-tiles. Column 128 of the accumulator is
    sum(exp) — normalize with DVE reciprocal + tensor_scalar multiply.
  - Causality: only k-tiles with k <= q are ever computed.
"""

import math

import numpy as np
import ml_dtypes

import concourse.tile as tile
import concourse.mybir as mybir
from concourse import bacc, bass_utils

B, H, S, F = 2, 16, 2048, 128
NCORES = 8
HPC = (B * H) // NCORES  # (b,h) pairs per core
SCALE = 1.0 / math.sqrt(F)
HALF = S // 2  # q processed in two 1024-wide halves (PSUM budget)
GSTRIDE = 136  # col stride of packed AV accumulator groups (32B aligned)

QK_FP32R = False  # scores/projection matmuls in fp32r (else bf16)

_cache = {}


def _build():
    f32 = mybir.dt.float32
    bf16 = mybir.dt.bfloat16
    f32r = mybir.dt.float32r
    qk_dt = f32r if QK_FP32R else bf16
    Exp = mybir.ActivationFunctionType.Exp

    nc = bacc.Bacc("TRN2")

    if QK_FP32R:
        xtq = nc.dram_tensor("xt32", [HPC, F, S], f32r, kind="ExternalInput")
    xtb = nc.dram_tensor("xtbh", [HPC, F, S], bf16, kind="ExternalInput")
    wqt = nc.dram_tensor("wqt", [HPC, F, F], qk_dt, kind="ExternalInput")
    wkt = nc.dram_tensor("wkt", [HPC, F, F], qk_dt, kind="ExternalInput")
    wvt = nc.dram_tensor("wvt", [HPC, F, F + 1], bf16, kind="ExternalInput")
    bqt = nc.dram_tensor("bqt", [F, HPC], f32, kind="ExternalInput")
    bkt = nc.dram_tensor("bkt", [F, HPC], f32, kind="ExternalInput")
    bvp = nc.dram_tensor("bvp", [1, HPC * (F + 1)], bf16, kind="ExternalInput")
    msk = nc.dram_tensor("msk", [F, F], bf16, kind="ExternalInput")
    one = nc.dram_tensor("one", [1, F], bf16, kind="ExternalInput")
    out = nc.dram_tensor("out", [HPC, S, F], f32, kind="ExternalOutput")
    if not QK_FP32R:
        xtq = xtb

    with tile.TileContext(nc) as tc, \
            tc.tile_pool(name="consts", bufs=1) as consts, \
            tc.tile_pool(name="xin", bufs=2) as xin, \
            tc.tile_pool(name="qk", bufs=2) as qkp, \
            tc.tile_pool(name="vp", bufs=2 * (S // F)) as vpp, \
            tc.tile_pool(name="pt", bufs=3) as ptp, \
            tc.tile_pool(name="outs", bufs=6) as outp, \
            tc.tile_pool(name="st", bufs=2, space="PSUM") as stp, \
            tc.tile_pool(name="av", bufs=3, space="PSUM") as avp, \
            tc.tile_pool(name="vq", bufs=1, space="PSUM") as vqp:

        c_bq = consts.tile([F, HPC], f32, tag="bq")
        nc.sync.dma_start(out=c_bq, in_=bqt[:, :])
        c_bk = consts.tile([F, HPC], f32, tag="bk")
        nc.sync.dma_start(out=c_bk, in_=bkt[:, :])
        c_bv = consts.tile([1, HPC * (F + 1)], bf16, tag="bv")
        nc.sync.dma_start(out=c_bv, in_=bvp[:, :])
        c_mask = consts.tile([F, F], bf16, tag="msk")
        nc.sync.dma_start(out=c_mask, in_=msk[:, :])
        c_one = consts.tile([1, F], bf16, tag="one")
        nc.sync.dma_start(out=c_one, in_=one[:, :])

        # deferred AV-batch emission: keeps PE busy with the next ST strip
        # while ACT runs exp on the current one
        pending = []

        def flush_pending():
            while pending:
                pending.pop(0)()

        def make_prelude(hd):
            """Emission closures for head hd's input DMAs, QT/KT and V'
            projections. Popped one-per-ki during head hd-1's k-loop so
            this work hides under the previous head's softmax."""
            st8 = {"vtiles": []}

            def dmas(hd=hd):
                if QK_FP32R:
                    x32 = xin.tile([F, S], f32r, tag="x32",
                                   name=f"x32_{hd}")
                    nc.sync.dma_start(out=x32, in_=xtq[hd])
                xbh = xin.tile([F, S], bf16, tag="xbh", name=f"xbh_{hd}")
                nc.sync.dma_start(out=xbh, in_=xtb[hd])
                wq = xin.tile([F, F], qk_dt, tag="wq", name=f"wq_{hd}")
                nc.sync.dma_start(out=wq, in_=wqt[hd])
                wk = xin.tile([F, F], qk_dt, tag="wk", name=f"wk_{hd}")
                nc.sync.dma_start(out=wk, in_=wkt[hd])
                wv = xin.tile([F, F + 1], bf16, tag="wv", name=f"wv_{hd}")
                nc.sync.dma_start(out=wv, in_=wvt[hd])
                st8["xbh"], st8["wq"], st8["wk"], st8["wv"] = xbh, wq, wk, wv
                st8["xqk"] = x32 if QK_FP32R else xbh
                st8["qt"] = qkp.tile([F, S], qk_dt, tag="qt",
                                     name=f"qt_{hd}")
                st8["kt"] = qkp.tile([F, S], qk_dt, tag="kt",
                                     name=f"kt_{hd}")

            def qk_chunk(which, c, hd=hd):
                wt, bt = ((st8["wq"], c_bq) if which == "q"
                          else (st8["wk"], c_bk))
                dst = st8["qt" if which == "q" else "kt"]
                ps = vqp.tile([128, 512], f32, tag="vq",
                              name=f"qk_{hd}_{which}{c}")
                nc.tensor.matmul(
                    ps[:, 0:512], wt[:, :],
                    st8["xqk"][:, 512 * c:512 * (c + 1)],
                    start=True, stop=True)
                nc.vector.tensor_scalar_add(
                    dst[:, 512 * c:512 * (c + 1)], ps[:, 0:512],
                    bt[:, hd:hd + 1])

            def vp_tile(si, hd=hd):
                ps = vqp.tile([128, 512], f32, tag="vq",
                              name=f"vps_{hd}_{si}")
                nc.tensor.matmul(
                    ps[:, 0:F + 1], c_one[:, :],
                    c_bv[:, (F + 1) * hd:(F + 1) * (hd + 1)],
                    start=True, stop=False)
                nc.tensor.matmul(
                    ps[:, 0:F + 1],
                    st8["xbh"][:, 128 * si:128 * (si + 1)], st8["wv"][:, :],
                    start=False, stop=True)
                vt = vpp.tile([128, F + 1], bf16, tag="vp",
                              name=f"vp_{hd}_{si}")
                nc.vector.tensor_copy(out=vt[:, :], in_=ps[:, 0:F + 1])
                st8["vtiles"].append(vt)

            closures = [dmas]
            for c in range(S // 512):
                closures.append(lambda c=c: qk_chunk("q", c))
                closures.append(lambda c=c: qk_chunk("k", c))
            for si in range(S // 128):
                closures.append(lambda si=si: vp_tile(si))
            return st8, closures

        head_state = {}
        head_state[0], prelude = make_prelude(0)
        while prelude:  # head 0 has no previous k-loop to hide under
            prelude.pop(0)()

        for hd in range(HPC):
            while prelude:  # leftovers from the previous k-loop
                prelude.pop(0)()
            if hd + 1 < HPC:
                head_state[hd + 1], prelude = make_prelude(hd + 1)
            qt_t = head_state[hd]["qt"]
            kt_t = head_state[hd]["kt"]
            vtiles = head_state[hd]["vtiles"]

            # --- attention, q in two 1024-wide halves ---
            for half in range(2):
                q0 = half * HALF
                nk = (half + 1) * (HALF // 128)  # k-tiles touching this half
                avts = [avp.tile([128, 512], f32, tag="av",
                                 name=f"avacc_{hd}_{half}_{i}")
                        for i in range(3)]
                # start=True clears has_written for the WHOLE bank, so a
                # per-group start would clobber the other groups packed in
                # the same bank. Clear each bank once with a dummy matmul
                # into a spare column; all real AV matmuls use start=False
                # (first write per element overwrites since its bit is clear).
                for b in range(3):
                    nc.tensor.matmul(
                        avts[b][:, 508:509], c_one[:, :], c_one[:, 0:1],
                        start=True, stop=False, skip_group_check=True)

                for ki in range(nk):
                    ks = 128 * ki
                    ls = max(0, ks - q0)  # local start col within strip
                    strip = stp.tile([128, 1024], f32, tag="st")
                    bounds = [ls, 512, 1024] if ls < 512 else [ls, 1024]
                    for c0, c1 in zip(bounds[:-1], bounds[1:]):
                        nc.tensor.matmul(
                            strip[:, c0:c1], kt_t[:, ks:ks + 128],
                            qt_t[:, q0 + c0:q0 + c1],
                            start=True, stop=True)
                    if prelude:  # hide next head's QKV/V' under this k-loop
                        prelude.pop(0)()
                    ptile = ptp.tile([128, 1024], bf16, tag="pt")
                    nc.scalar.activation(
                        out=ptile[:, ls:1024], in_=strip[:, ls:1024],
                        func=Exp, scale=SCALE)
                    if ks >= q0:  # zero the below-diagonal of the diag block
                        nc.vector.tensor_mul(
                            ptile[:, ls:ls + 128], ptile[:, ls:ls + 128],
                            c_mask[:, :])

                    def av_batch(hd=hd, half=half, ki=ki, ptile=ptile,
                                 avts=avts, vtiles=vtiles):
                        for qt in range(max(0, ki - 8 * half), 8):
                            qg = 8 * half + qt
                            g = GSTRIDE * (qt % 3)
                            acc = avts[qt // 3][:, g:g + F + 1]
                            nc.tensor.matmul(
                                acc, ptile[:, 128 * qt:128 * qt + 128],
                                vtiles[ki][:, :],
                                start=False, stop=(ki == qg),
                                skip_group_check=True)
                        # normalize + store once a whole accumulator bank
                        # is finished (avoids PE-write/DVE-read bank overlap)
                        for bank in range(3):
                            last_qt = min(3 * bank + 2, 7)
                            if ki != 8 * half + last_qt:
                                continue
                            for qt in range(3 * bank, last_qt + 1):
                                qg = 8 * half + qt
                                g = GSTRIDE * (qt % 3)
                                acc = avts[bank][:, g:g + F + 1]
                                rc = outp.tile([128, 1], f32, tag="rc")
                                nc.vector.reciprocal(rc[:, :], acc[:, F:F + 1])
                                ot = outp.tile([128, F], f32, tag="ot")
                                nc.vector.tensor_scalar_mul(
                                    ot[:, :], acc[:, 0:F], rc[:, :])
                                nc.sync.dma_start(
                                    out=out[hd, 128 * qg:128 * (qg + 1), :],
                                    in_=ot[:, :])

                    flush_pending()
                    pending.append(av_batch)
        flush_pending()

    nc.compile()
    return nc


def _prep_inputs(x, Wq, Wk, Wv, bq, bk, bv):
    """Shard + pre-transpose on host. Returns in_maps for 8 cores."""
    bf16 = ml_dtypes.bfloat16
    xf = np.ascontiguousarray(
        x.reshape(B * H, S, F).transpose(0, 2, 1)).astype(np.float32)  # [32,F,S]
    xfb = xf.astype(bf16)
    wqT = np.ascontiguousarray(Wq.transpose(0, 2, 1)).astype(np.float32)  # [H,f,e]
    wkT = np.ascontiguousarray(Wk.transpose(0, 2, 1)).astype(np.float32)
    wvT = np.ascontiguousarray(Wv.transpose(0, 2, 1)).astype(np.float32)
    wvTp = np.zeros((H, F, F + 1), np.float32)
    wvTp[:, :, :F] = wvT
    wvTp = wvTp.astype(bf16)
    bvp_h = np.concatenate(
        [bv.astype(np.float32), np.ones((H, 1), np.float32)], axis=1)  # [H,129]
    mask = np.triu(np.ones((F, F), np.float32)).astype(bf16)  # keep r <= c
    ones_row = np.ones((1, F), np.float32).astype(bf16)

    wq_dt = np.float32 if QK_FP32R else bf16
    in_maps = []
    for c in range(NCORES):
        pairs = list(range(HPC * c, HPC * (c + 1)))
        heads = [p % H for p in pairs]
        m = {
            "xtbh": np.ascontiguousarray(xfb[pairs]),
            "wqt": np.ascontiguousarray(wqT[heads]).astype(wq_dt),
            "wkt": np.ascontiguousarray(wkT[heads]).astype(wq_dt),
            "wvt": np.ascontiguousarray(wvTp[heads]),
            "bqt": np.ascontiguousarray(bq[heads].T).astype(np.float32),
            "bkt": np.ascontiguousarray(bk[heads].T).astype(np.float32),
            "bvp": np.ascontiguousarray(
                bvp_h[heads].reshape(1, HPC * (F + 1))).astype(bf16),
            "msk": mask,
            "one": ones_row,
        }
        if QK_FP32R:
            m["xt32"] = np.ascontiguousarray(xf[pairs])
        in_maps.append(m)
    return in_maps


def kernel(x, Wq, Wk, Wv, bq, bk, bv, trace=False):
    x, Wq, Wk, Wv = (np.asarray(a, np.float32) for a in (x, Wq, Wk, Wv))
    bq, bk, bv = (np.asarray(a, np.float32) for a in (bq, bk, bv))

    if "nc" not in _cache:
        _cache["nc"] = _build()
    nc = _cache["nc"]

    in_maps = _prep_inputs(x, Wq, Wk, Wv, bq, bk, bv)
    res = bass_utils.run_bass_kernel_spmd(
        nc, in_maps, core_ids=list(range(NCORES)), trace=trace)

    out = np.empty((B * H, S, F), np.float32)
    for c in range(NCORES):
        out[HPC * c:HPC * (c + 1)] = res.results[c]["out"]
    full = out.reshape(B, H, S, F)
    if trace:
        return full, res
    return full
